# Initial kernel scaffold
#
"""BitConv2d (BitNet-style fake-quant 3x3 conv) Trainium2 Bass kernel.

Reference computation (see problem):
  ws   = max(mean|w|, 1e-6);  qw = clip(round(w/ws), -1, 1)           (per-tensor ternary)
  amax = max(max|x| over (N,H,W) per channel, 1e-6); xs = 127/amax
  qx   = clip(round(x*xs), -128, 127)                                  (per-channel int8)
  out  = conv2d(qx/xs, qw*ws, stride 1, pad 1, NCHW/OIHW) + bias

Key algebraic restructuring for the tensor engine:
  out[n,o,h,w] = sum_{c,i,j} qx[n,c,h+i-1,w+j-1] * (qw[o,c,i,j] * ws * amax[c]/127)
so the conv runs as bf16 matmuls with
  rhs  = qx          (integers in [-127,127]  -> EXACT in bf16)
  lhsT = qw * s_c    (ternary * per-in-channel scale, bf16-rounded once per channel)
accumulated in fp32 PSUM. The 3x3 conv is 18 accumulating matmuls
(2 cin-tiles x 9 taps) over a zero-padded flat spatial layout where each tap
is a constant column offset (di*58+dj).

Sharding: data-parallel over batch (4 images/core on 8 cores), weight
replicated (ws computed redundantly); per-channel amax needs a global max
-> tiny in-kernel AllReduce(max) of 256 floats across the 8 cores.
"""

import sys
import types

for _p in ("/opt/trn_rl_repo", "/root/.axon_site/_ro/trn_rl_repo"):
    if _p not in sys.path:
        sys.path.insert(0, _p)

import numpy as np
import ml_dtypes

import concourse.bacc as bacc
import concourse.mybir as mybir
import concourse.tile as tile
from concourse.bass_utils import run_bass_kernel_spmd

F32 = mybir.dt.float32
BF16 = mybir.dt.bfloat16
ALU = mybir.AluOpType
AX = mybir.AxisListType

N_CORES = 8
N, CIN, H, W = 32, 256, 56, 56
COUT, KH, KW = 256, 3, 3
NPC = N // N_CORES          # images per core
HW = H * W                  # 3136
PW = W + 2                  # 58: padded width (flat layout row stride)
QCOLS = 3368                # padded qx cols: >= 56*58 + 2*58 + 2 = 3366, 8-aligned
ROWS_PER_CHUNK = 8
CHUNK = ROWS_PER_CHUNK * PW   # 464 psum cols per chunk (<=512, one bank)
NCHUNK = H // ROWS_PER_CHUNK  # 7
OUT_CHUNK = ROWS_PER_CHUNK * W  # 448 valid cols per chunk
MAGIC = 12582912.0          # 1.5*2^23: (v+MAGIC)-MAGIC == round-half-even(v)
EPS = 1e-6
FAN = COUT * CIN * KH * KW  # weight element count for mean|w|


def _build_program():
    nc = bacc.Bacc(
        "TRN2",
        target_bir_lowering=False,
        debug=False,
        enable_asserts=False,
        num_devices=N_CORES,
    )
    x_d = nc.dram_tensor("x", [NPC, CIN, H, W], F32, kind="ExternalInput")
    w_d = nc.dram_tensor("weight", [COUT, CIN, KH, KW], F32, kind="ExternalInput")
    b_d = nc.dram_tensor("bias", [COUT], F32, kind="ExternalInput")
    o_d = nc.dram_tensor("out", [NPC, COUT, H, W], F32, kind="ExternalOutput")
    ident_d = nc.inline_tensor(np.eye(128, dtype=ml_dtypes.bfloat16), name="ident")

    x_flat = x_d.ap().rearrange("n c h w -> n c (h w)")
    o_flat = o_d.ap().rearrange("n c h w -> n c (h w)")
    w_flat = w_d.ap().rearrange("o c kh kw -> o (c kh kw)")  # free idx = c*9 + tap

    with tile.TileContext(nc) as tc:
        with tc.tile_pool(name="persist", bufs=1) as pp, \
             tc.tile_pool(name="dram", bufs=1, space="DRAM") as dram:
            # ---- persistent tiles ----
            xt = [pp.tile([128, HW], F32, name=f"xt{i}") for i in range(NPC * 2)]
            qx = [pp.tile([128, QCOLS], BF16, name=f"qx{i}") for i in range(NPC * 2)]
            # 36 weight tiles; idx = ct*18 + ot*9 + tap
            qwT = pp.tile([128, 36, 128], BF16, name="qwT")    # unscaled ternary^T
            lhsT = pp.tile([128, 36, 128], BF16, name="lhsT")  # scaled, matmul-ready
            ident_sb = pp.tile([128, 128], BF16, name="ident_sb")
            bias_sb = pp.tile([128, 2], F32, name="bias_sb")
            wsb = pp.tile([128, 2], F32, name="wsb")       # col0 = ws, col1 = 1/ws
            ws1 = pp.tile([1, 2], F32, name="ws1")
            ones_k = pp.tile([128, 1], F32, name="ones_k")
            ones_m = pp.tile([1, 128], F32, name="ones_m")
            pamax = pp.tile([128, 8], F32, name="pamax")   # per (n,ct) partial amax
            amax2 = pp.tile([128, 2], F32, name="amax2")   # local then global amax
            xs = pp.tile([128, 2], F32, name="xs")         # 127/amax
            sc = pp.tile([128, 2], F32, name="sc")         # ws*amax/127
            cc_in = dram.tile([128, 2], F32, name="cc_in")
            cc_out = dram.tile([128, 2], F32, name="cc_out", addr_space="Shared")

            # ---- zero-fill qx padding, load constants, start x loads ----
            for i in range(NPC * 2):
                nc.gpsimd.memset(qx[i][:], 0.0)
            nc.sync.dma_start(ident_sb[:], ident_d.ap())
            nc.sync.dma_start(bias_sb[:], b_d.ap().rearrange("(o p) -> p o", p=128))
            for n in range(NPC):
                for ct in range(2):
                    nc.sync.dma_start(
                        xt[n * 2 + ct][:],
                        x_flat[n, ct * 128:(ct + 1) * 128, :],
                    )

            # ---- weight prep (independent of x / collective) ----
            with tc.tile_pool(name="wtmp", bufs=2) as wp, \
                 tc.tile_pool(name="psum_t", bufs=4, space="PSUM") as pt_pool, \
                 tc.tile_pool(name="psum_s", bufs=2, space="PSUM") as ps_pool:
                wnat = [wp.tile([128, CIN * 9], F32, name=f"wnat{ot}", tag=f"wnat{ot}")
                        for ot in range(2)]
                absw = wp.tile([128, 2], F32, name="absw", tag="absw")
                rowsum = wp.tile([128, 1], F32, name="rowsum", tag="rowsum")
                for ot in range(2):
                    nc.sync.dma_start(wnat[ot][:], w_flat[ot * 128:(ot + 1) * 128, :])
                # ws = mean|w| (all cores compute this redundantly)
                for ot in range(2):
                    nc.vector.reduce_sum(absw[:, ot:ot + 1], wnat[ot][:],
                                         axis=AX.X, apply_absolute_value=True)
                nc.vector.tensor_add(rowsum[:], absw[:, 0:1], absw[:, 1:2])
                nc.vector.memset(ones_k[:], 1.0)
                nc.vector.memset(ones_m[:], 1.0)
                ps_s = ps_pool.tile([1, 1], F32, name="ps_s")
                nc.tensor.matmul(ps_s[:], ones_k[:], rowsum[:], start=True, stop=True)
                nc.vector.tensor_scalar(ws1[0:1, 0:1], ps_s[:], 1.0 / FAN, EPS,
                                        op0=ALU.mult, op1=ALU.max)
                nc.vector.reciprocal(ws1[0:1, 1:2], ws1[0:1, 0:1])
                ps_b = ps_pool.tile([128, 2], F32, name="ps_b")
                nc.tensor.matmul(ps_b[:], ones_m[:], ws1[0:1, :], start=True, stop=True)
                nc.scalar.copy(wsb[:], ps_b[:])

                # ternary quantize: qw = clip(round(w * (1/ws)), -1, 1)
                for ot in range(2):
                    t = wnat[ot]
                    nc.vector.tensor_scalar(t[:], t[:], wsb[:, 1:2], MAGIC,
                                            op0=ALU.mult, op1=ALU.add)
                    nc.vector.tensor_scalar_sub(t[:], t[:], MAGIC)
                    nc.vector.tensor_scalar(t[:], t[:], -1.0, 1.0,
                                            op0=ALU.max, op1=ALU.min)
                # transpose each [o,c] 128x128 block per tap -> qwT[c, o]
                wv = [wnat[ot].rearrange("p (c t) -> p t c", t=9) for ot in range(2)]
                for ct in range(2):
                    for ot in range(2):
                        for tap in range(9):
                            idx = ct * 18 + ot * 9 + tap
                            pt = pt_pool.tile([128, 128], F32, name="pt", tag="pt")
                            nc.tensor.transpose(
                                pt[:],
                                wv[ot][:, tap, ct * 128:(ct + 1) * 128],
                                ident_sb[:],
                            )
                            nc.scalar.copy(qwT[:, idx, :], pt[:])

            # ---- per-channel |x| max over local images ----
            for n in range(NPC):
                for ct in range(2):
                    i = n * 2 + ct
                    nc.vector.reduce_max(pamax[:, i:i + 1], xt[i][:],
                                         axis=AX.X, apply_absolute_value=True)
            pv = pamax.rearrange("p (n c) -> p c n", c=2)
            for ct in range(2):
                nc.vector.reduce_max(amax2[:, ct:ct + 1], pv[:, ct, :], axis=AX.X)

            # ---- global amax: AllReduce(max) of 256 floats across 8 cores ----
            nc.sync.dma_start(cc_in[:], amax2[:])
            nc.gpsimd.collective_compute(
                "AllReduce", ALU.max,
                replica_groups=[list(range(N_CORES))],
                ins=[cc_in.opt()], outs=[cc_out.opt()],
            )
            nc.sync.dma_start(amax2[:], cc_out[:])

            # ---- scales ----
            nc.vector.tensor_scalar_max(amax2[:], amax2[:], EPS)
            nc.vector.reciprocal(xs[:], amax2[:])
            nc.vector.tensor_scalar_mul(xs[:], xs[:], 127.0)
            nc.vector.tensor_scalar(sc[:], amax2[:], wsb[:, 0:1], 1.0 / 127.0,
                                    op0=ALU.mult, op1=ALU.mult)
            for ct in range(2):
                nc.vector.tensor_scalar_mul(
                    lhsT[:, ct * 18:(ct + 1) * 18, :],
                    qwT[:, ct * 18:(ct + 1) * 18, :],
                    sc[:, ct:ct + 1],
                )

            # ---- quantize x + conv ----
            with tc.tile_pool(name="psum_c", bufs=6, space="PSUM") as pc_pool, \
                 tc.tile_pool(name="outp", bufs=6) as op_pool:
                for n in range(NPC):
                    for ct in range(2):
                        i = n * 2 + ct
                        t = xt[i]
                        # qx = round(x * xs)  (integer-valued, exact in bf16)
                        nc.vector.tensor_scalar(t[:], t[:], xs[:, ct:ct + 1], MAGIC,
                                                op0=ALU.mult, op1=ALU.add)
                        qxv = qx[i][:, 59:59 + H * PW].rearrange(
                            "p (h w) -> p h w", w=PW)[:, :, 0:W]
                        nc.vector.tensor_scalar_sub(
                            qxv, t.rearrange("p (h w) -> p h w", w=W), MAGIC)
                    for ot in range(2):
                        for c8 in range(NCHUNK):
                            ps = pc_pool.tile([128, 512], F32, name="ps", tag="ps")
                            base = c8 * CHUNK
                            k = 0
                            for ct in range(2):
                                for tap in range(9):
                                    di, dj = tap // 3, tap % 3
                                    off = base + di * PW + dj
                                    nc.tensor.matmul(
                                        ps[:, 0:CHUNK],
                                        lhsT[:, ct * 18 + ot * 9 + tap, :],
                                        qx[n * 2 + ct][:, off:off + CHUNK],
                                        start=(k == 0), stop=(k == 17),
                                    )
                                    k += 1
                            ob = op_pool.tile([128, OUT_CHUNK], F32, name="ob", tag="ob")
                            nc.vector.tensor_scalar_add(
                                ob.rearrange("p (h w) -> p h w", w=W),
                                ps[:, 0:CHUNK].rearrange(
                                    "p (h w) -> p h w", w=PW)[:, :, 0:W],
                                bias_sb[:, ot:ot + 1],
                            )
                            nc.sync.dma_start(
                                o_flat[n, ot * 128:(ot + 1) * 128,
                                       c8 * OUT_CHUNK:(c8 + 1) * OUT_CHUNK],
                                ob[:],
                            )

    nc.compile()
    return nc


_NC_CACHE = None


def _get_program():
    global _NC_CACHE
    if _NC_CACHE is None:
        _NC_CACHE = _build_program()
    return _NC_CACHE


def _install_ntff_hook():
    """Register the axon NTFF profiling hook (the antenv stub lacks it)."""
    try:
        import antenv
        if getattr(antenv, "axon_hooks", None) is not None:
            return
        mod = types.ModuleType("antenv.axon_hooks")
        mod._hook = None
        def set_axon_ntff_profile_hook(h):
            mod._hook = h
        def get_axon_ntff_profile_hook():
            return mod._hook
        mod.set_axon_ntff_profile_hook = set_axon_ntff_profile_hook
        mod.get_axon_ntff_profile_hook = get_axon_ntff_profile_hook
        sys.modules["antenv.axon_hooks"] = mod
        antenv.axon_hooks = mod
        from trn_agent_boot.trn_boot import _ntff_profile_via_ctypes
        set_axon_ntff_profile_hook(_ntff_profile_via_ctypes("/opt/axon/libaxon_pjrt.so"))
    except Exception:
        pass


def run(x, weight, bias, trace=False):
    x = np.ascontiguousarray(np.asarray(x, dtype=np.float32))
    weight = np.ascontiguousarray(np.asarray(weight, dtype=np.float32))
    bias = np.ascontiguousarray(np.asarray(bias, dtype=np.float32))
    assert x.shape == (N, CIN, H, W), x.shape
    nc = _get_program()
    in_maps = [
        {"x": x[c * NPC:(c + 1) * NPC], "weight": weight, "bias": bias}
        for c in range(N_CORES)
    ]
    if trace:
        _install_ntff_hook()
    res = run_bass_kernel_spmd(nc, in_maps, list(range(N_CORES)), trace=trace)
    out = np.concatenate([res.results[c]["out"] for c in range(N_CORES)], axis=0)
    return out, res


def kernel(x, weight, bias):
    out, _ = run(x, weight, bias, trace=False)
    return out


# revision 25
# speedup vs baseline: 1.0365x; 1.0365x over previous
"""BitConv2d (BitNet-style fake-quant 3x3 conv) Trainium2 Bass kernel.

Reference computation (see problem):
  ws   = max(mean|w|, 1e-6);  qw = clip(round(w/ws), -1, 1)           (per-tensor ternary)
  amax = max(max|x| over (N,H,W) per channel, 1e-6); xs = 127/amax
  qx   = clip(round(x*xs), -128, 127)                                  (per-channel int8)
  out  = conv2d(qx/xs, qw*ws, stride 1, pad 1, NCHW/OIHW) + bias

Key algebraic restructuring for the tensor engine:
  out[n,o,h,w] = sum_{c,i,j} qx[n,c,h+i-1,w+j-1] * (qw[o,c,i,j] * ws * amax[c]/127)
so the conv runs as bf16 matmuls with
  rhs  = qx          (integers in [-127,127]  -> EXACT in bf16)
  lhsT = qw * s_c    (ternary * per-in-channel scale, bf16-rounded once per channel)
accumulated in fp32 PSUM. The 3x3 conv is 18 accumulating matmuls
(2 cin-tiles x 9 taps) over a zero-padded flat spatial layout with row
stride 57 (one left-pad column per row doubles as the previous row's right
pad), where each tap is a constant flat column offset di*57+dj.

Sharding: data-parallel over batch (4 images/core on 8 cores), weight
replicated (ws computed redundantly); per-channel amax needs a global max
-> tiny in-kernel AllGather of the 8 partial [256] maxima + local reduce.
Pass-B x re-loads are dependency-gated behind the collective so the
collective's SDMA traffic runs on a quiet fabric.
"""

import sys
import types

for _p in ("/opt/trn_rl_repo", "/root/.axon_site/_ro/trn_rl_repo"):
    if _p not in sys.path:
        sys.path.insert(0, _p)

import numpy as np
import ml_dtypes

import concourse.bacc as bacc
import concourse.mybir as mybir
import concourse.tile as tile
from concourse.bass_utils import run_bass_kernel_spmd
from concourse.tile_rust import add_dep_helper

F32 = mybir.dt.float32
BF16 = mybir.dt.bfloat16
ALU = mybir.AluOpType
AX = mybir.AxisListType
AF = mybir.ActivationFunctionType

N_CORES = 8
N, CIN, H, W = 32, 256, 56, 56
COUT, KH, KW = 256, 3, 3
NPC = N // N_CORES          # images per core
HW = H * W                  # 3136
PW = W + 1                  # 57: padded row stride (left pad doubles as right pad)
QCOLS = 3312                # >= (55+2)*57 + 58 = 3307, 8-aligned
ROWS_PER_CHUNK = 8
CHUNK = ROWS_PER_CHUNK * PW   # 456 psum cols per chunk (<=512, one bank)
NCHUNK = H // ROWS_PER_CHUNK  # 7
OUT_CHUNK = ROWS_PER_CHUNK * W  # 448 valid cols per chunk
MAGIC = 12582912.0          # 1.5*2^23: (v+MAGIC)-MAGIC == round-half-even(v)
EPS = 1e-6
FAN = COUT * CIN * KH * KW  # weight element count for mean|w|


def _build_program():
    nc = bacc.Bacc(
        "TRN2",
        target_bir_lowering=False,
        debug=False,
        enable_asserts=False,
        num_devices=N_CORES,
    )
    x_d = nc.dram_tensor("x", [NPC, CIN, H, W], F32, kind="ExternalInput")
    w_d = nc.dram_tensor("weight", [COUT, CIN, KH, KW], F32, kind="ExternalInput")
    b_d = nc.dram_tensor("bias", [COUT], F32, kind="ExternalInput")
    o_d = nc.dram_tensor("out", [NPC, COUT, H, W], F32, kind="ExternalOutput")
    ident_d = nc.inline_tensor(np.eye(128, dtype=ml_dtypes.bfloat16), name="ident")

    x_flat = x_d.ap().rearrange("n c h w -> n c (h w)")
    o_flat = o_d.ap().rearrange("n c h w -> n c (h w)")
    w_flat = w_d.ap().rearrange("o c kh kw -> o (c kh kw)")  # free idx = c*9 + tap

    with tile.TileContext(nc) as tc:
        with tc.tile_pool(name="persist", bufs=1) as pp, \
             tc.tile_pool(name="xstream", bufs=3) as xsp, \
             tc.tile_pool(name="dram", bufs=1, space="DRAM") as dram:
            # ---- persistent tiles ----
            qx = [pp.tile([128, QCOLS], BF16, name=f"qx{i}") for i in range(NPC * 2)]
            # 36 weight tiles; idx = ct*18 + ot*9 + tap; scaled in place post-CC
            lhsT = pp.tile([128, 36, 128], BF16, name="lhsT")
            ident_sb = pp.tile([128, 128], BF16, name="ident_sb")
            # all small scalars packed into one tile (slots are 4KB-padded)
            misc = pp.tile([128, 160], F32, name="misc")
            ones_m = misc[0:1, 0:128]
            ones_k = misc[:, 128:129]
            bias_sb = misc[:, 130:132]
            wsb = misc[:, 132:134]     # col0 = ws, col1 = 1/ws
            xs = misc[:, 134:136]      # 127/amax
            sc = misc[:, 136:138]      # ws*amax/127
            amax2 = misc[:, 138:140]
            pamax = misc[:, 140:148]   # per (n,ct) partial amax
            ws1 = misc[0:1, 148:150]
            absw = misc[:, 150:152]
            cc_in = dram.tile([128, 2], F32, name="cc_in")
            cc_out = dram.tile([128, 2], F32, name="cc_out",
                               addr_space="Shared")


            # ---- zero-fill qx padding, load constants ----
            for i in range(NPC * 2):
                nc.gpsimd.memset(qx[i][:], 0.0)
            nc.sync.dma_start(ident_sb[:], ident_d.ap())
            nc.sync.dma_start(bias_sb, b_d.ap().rearrange("(o p) -> p o", p=128))
            nc.vector.memset(ones_k, 1.0)
            nc.vector.memset(ones_m, 1.0)

            with tc.tile_pool(name="wtmp", bufs=1) as wp, \
                 tc.tile_pool(name="psum_t", bufs=4, space="PSUM") as pt_pool, \
                 tc.tile_pool(name="psum_s", bufs=1, space="PSUM") as ps_pool:
                # ---- pass A: stream x on Sync DMA; per-(n,ct) |x| max.
                # The last image's tiles stay resident for pass B.
                xres = {}
                for n in range(NPC):
                    for ct in range(2):
                        t = xsp.tile([128, HW], F32, name="xa", tag="xa")
                        nc.sync.dma_start(t[:],
                                          x_flat[n, ct * 128:(ct + 1) * 128, :])
                        nc.vector.reduce_max(
                            pamax[:, n * 2 + ct:n * 2 + ct + 1], t[:],
                            axis=AX.X, apply_absolute_value=True)
                        xres[(n, ct)] = t
                # local amax over images, kick off the collective immediately
                pv = pamax.rearrange("p (n c) -> p c n", c=2)
                for ct in range(2):
                    nc.vector.reduce_max(amax2[:, ct:ct + 1], pv[:, ct, :],
                                         axis=AX.X)
                nc.gpsimd.dma_start(cc_in[:], amax2)
                nc.gpsimd.collective_compute(
                    "AllReduce", ALU.max,
                    replica_groups=[list(range(N_CORES))],
                    ins=[cc_in.opt()], outs=[cc_out.opt()],
                )
                cc_ret = nc.gpsimd.dma_start(amax2, cc_out[:])

                # ---- weight prep (Sync DMA queued behind pass A so it never
                # steals bandwidth from the amax-critical x stream)
                wt1 = []
                for ot in range(2):
                    wt = wp.tile([128, CIN * 9], F32, name=f"wt{ot}", tag=f"wt{ot}")
                    nc.sync.dma_start(wt[:], w_flat[ot * 128:(ot + 1) * 128, :])
                    wt1.append(wt)
                for ot in range(2):
                    nc.vector.reduce_sum(absw[:, ot:ot + 1], wt1[ot][:],
                                         axis=AX.X, apply_absolute_value=True)
                nc.vector.tensor_add(absw[:, 0:1], absw[:, 0:1], absw[:, 1:2])
                ps_s = ps_pool.tile([1, 1], F32, name="ps_s")
                nc.tensor.matmul(ps_s[:], ones_k, absw[:, 0:1], start=True, stop=True)
                nc.vector.tensor_scalar(ws1[:, 0:1], ps_s[:], 1.0 / FAN, EPS,
                                        op0=ALU.mult, op1=ALU.max)
                nc.vector.reciprocal(ws1[:, 1:2], ws1[:, 0:1])
                ps_b = ps_pool.tile([128, 2], F32, name="ps_b")
                nc.tensor.matmul(ps_b[:], ones_m, ws1[:, :], start=True, stop=True)
                nc.scalar.copy(wsb, ps_b[:])

                # ternary quantize qw = clip(round(w/ws), -1, 1) in place, then
                # PE-transpose each [o,c] 128x128 block per tap -> lhsT[c, o]
                for ot in range(2):
                    wt = wt1[ot]
                    nc.vector.tensor_scalar(wt[:], wt[:], wsb[:, 1:2], MAGIC,
                                            op0=ALU.mult, op1=ALU.add)
                    nc.vector.tensor_scalar_sub(wt[:], wt[:], MAGIC)
                    qwb = wp.tile([128, CIN * 9], BF16, name="qwb", tag="qwb",
                                  bufs=2)
                    nc.vector.tensor_scalar(qwb[:], wt[:], -1.0, 1.0,
                                            op0=ALU.max, op1=ALU.min)
                    wv = qwb.rearrange("p (c t) -> p t c", t=9)
                    for ct in range(2):
                        for tap in range(9):
                            idx = ct * 18 + ot * 9 + tap
                            pt = pt_pool.tile([128, 128], BF16, name="pt", tag="pt")
                            nc.tensor.transpose(
                                pt[:],
                                wv[:, tap, ct * 128:(ct + 1) * 128],
                                ident_sb[:],
                            )
                            nc.scalar.copy(lhsT[:, idx, :], pt[:])

                # ---- scales. The per-channel dequant scale s_c = ws*amax/127
                # is folded into the quantize epilogue (rhs side), so lhsT
                # stays exact ternary bf16. ----
                nc.vector.tensor_scalar_max(amax2, amax2, EPS)
                nc.vector.reciprocal(xs, amax2)
                nc.vector.tensor_scalar_mul(xs, xs, 127.0)
                nc.vector.tensor_scalar(sc, amax2, wsb[:, 0:1], 1.0 / 127.0,
                                        op0=ALU.mult, op1=ALU.mult)


                # ---- pass B: quantize all images up front. Image 3 uses the
                # resident pass-A tiles; the rest re-stream, gated behind the
                # collective so its SDMA runs on a quiet fabric. ----
                first_xb = None
                for n in [3, 2, 1, 0]:
                    for ct in range(2):
                        i = n * 2 + ct
                        if n == NPC - 1:
                            t = xres[(n, ct)]
                        else:
                            t = xsp.tile([128, HW], F32, name="xb", tag="xa")
                            d = nc.sync.dma_start(
                                t[:], x_flat[n, ct * 128:(ct + 1) * 128, :])
                            if first_xb is None:
                                first_xb = d
                                add_dep_helper(d.ins, cc_ret.ins,
                                               reason="quiet fabric for CC")
                        # qx' = round(x*xs) * s_c: the subtract is exact at any
                        # ALU precision, then one bf16 rounding of qx*s_c.
                        nc.vector.tensor_scalar(t[:], t[:], xs[:, ct:ct + 1],
                                                MAGIC, op0=ALU.mult, op1=ALU.add)
                        qxv = qx[i][:, PW + 1:PW + 1 + H * PW].rearrange(
                            "p (h w) -> p h w", w=PW)[:, :, 0:W]
                        nc.vector.tensor_scalar(
                            qxv, t.rearrange("p (h w) -> p h w", w=W),
                            MAGIC, sc[:, ct:ct + 1],
                            op0=ALU.subtract, op1=ALU.mult)

            # ---- conv: weight-stationary over 7 chunk-banks per (image, ot) ----
            with tc.tile_pool(name="psum_c", bufs=8, space="PSUM") as pc_pool, \
                 tc.tile_pool(name="outp", bufs=6) as op_pool:
                for n in [3, 2, 1, 0]:
                    for ot in range(2):
                        pss = [pc_pool.tile([128, 512], F32, name="ps", tag="ps")
                               for _ in range(NCHUNK)]
                        k = 0
                        for ct in range(2):
                            for tap in range(9):
                                di, dj = tap // 3, tap % 3
                                lw = lhsT[:, ct * 18 + ot * 9 + tap, :]
                                qxi = qx[n * 2 + ct]
                                for c8 in range(NCHUNK):
                                    off = c8 * CHUNK + di * PW + dj
                                    nc.tensor.matmul(
                                        pss[c8][:, 0:CHUNK], lw,
                                        qxi[:, off:off + CHUNK],
                                        start=(k == 0), stop=(k == 17),
                                    )
                                k += 1
                        for c8 in range(NCHUNK):
                            ob = op_pool.tile([128, OUT_CHUNK], F32,
                                              name="ob", tag="ob")
                            nc.scalar.activation(
                                ob.rearrange("p (h w) -> p h w", w=W),
                                pss[c8][:, 0:CHUNK].rearrange(
                                    "p (h w) -> p h w", w=PW)[:, :, 0:W],
                                AF.Identity, bias=bias_sb[:, ot:ot + 1])
                            nc.sync.dma_start(
                                o_flat[n, ot * 128:(ot + 1) * 128,
                                       c8 * OUT_CHUNK:(c8 + 1) * OUT_CHUNK],
                                ob[:],
                            )

    nc.compile()
    return nc


_NC_CACHE = None


def _get_program():
    global _NC_CACHE
    if _NC_CACHE is None:
        _NC_CACHE = _build_program()
    return _NC_CACHE


def _install_ntff_hook():
    """Register the axon NTFF profiling hook (the antenv stub lacks it)."""
    try:
        import antenv
        if getattr(antenv, "axon_hooks", None) is not None:
            return
        mod = types.ModuleType("antenv.axon_hooks")
        mod._hook = None
        def set_axon_ntff_profile_hook(h):
            mod._hook = h
        def get_axon_ntff_profile_hook():
            return mod._hook
        mod.set_axon_ntff_profile_hook = set_axon_ntff_profile_hook
        mod.get_axon_ntff_profile_hook = get_axon_ntff_profile_hook
        sys.modules["antenv.axon_hooks"] = mod
        antenv.axon_hooks = mod
        from trn_agent_boot.trn_boot import _ntff_profile_via_ctypes
        set_axon_ntff_profile_hook(_ntff_profile_via_ctypes("/opt/axon/libaxon_pjrt.so"))
    except Exception:
        pass


def run(x, weight, bias, trace=False):
    x = np.ascontiguousarray(np.asarray(x, dtype=np.float32))
    weight = np.ascontiguousarray(np.asarray(weight, dtype=np.float32))
    bias = np.ascontiguousarray(np.asarray(bias, dtype=np.float32))
    assert x.shape == (N, CIN, H, W), x.shape
    nc = _get_program()
    in_maps = [
        {"x": x[c * NPC:(c + 1) * NPC], "weight": weight, "bias": bias}
        for c in range(N_CORES)
    ]
    if trace:
        _install_ntff_hook()
    res = run_bass_kernel_spmd(nc, in_maps, list(range(N_CORES)), trace=trace)
    out = np.concatenate([res.results[c]["out"] for c in range(N_CORES)], axis=0)
    return out, res


def kernel(x, weight, bias):
    out, _ = run(x, weight, bias, trace=False)
    return out
